# revision 18
# baseline (speedup 1.0000x reference)
"""Cross-attention Trainium2 kernel (8 NeuronCores), v2.

Sharding: batch (2) x head-groups (4 groups of 4 heads) = 8 shards.
Each core computes q/k/v projections for its 4 heads (256 cols of
Wq/Wk/Wv), attention for those heads, and a partial out-projection
through its 256 rows of Wo.  The host sums the 4 partial outputs per
batch and adds the bv @ Wo + bo correction (commutes through the
softmax average).

v2 redesign (vs v1): the PE *sequencer* was the bottleneck (3840 PE
instructions x ~58ns ~= the whole 224us span; PE engine only 70%
busy).  Changes:
  - inputs are transposed on the HOST (xT/cT in DRAM), eliminating all
    512 PE transpose instructions and the xtw staging copies.
  - attention runs "flipped": stationary = V chunks [sk128, 65] (65th
    col = ones -> denominator row 64), moving = exp tiles -> psum
    attnT [65, sq].  16 wide matmuls per (head, sq-half) instead of
    128 narrow ones; output lands directly in the [head_dim, sq]
    layout that out_proj consumes as its stationary operand, so the
    old per-chunk attn transposes disappear too.
  - softmax normalization: DVE reciprocal of the psum denominator row,
    then one DVE tensor_tensor multiply with a partition-broadcast AP.
  - out_proj DMAs straight from PSUM.
Total PE instructions: ~1664 (832 matmuls, self-loading), PE engine
~164us is the model critical path, ACT (exp) ~123us just under it.
"""

import numpy as np

import concourse.bass as bass
import concourse.mybir as mybir
import concourse.tile as tile
from concourse import bacc

B, SQ, SK, D, H, HS = 2, 2048, 2048, 1024, 16, 64
SCALE = HS ** -0.5
NCORES = 8
HG = 4            # heads per core
DG = HG * HS      # 256 projection cols per core

F32 = mybir.dt.float32
F32R = mybir.dt.float32r
F16 = mybir.dt.float16


def build_program(fast_mm: bool = True, pipeline: bool = True, loop_iters: int = 0):
    nc = bacc.Bacc(None, target_bir_lowering=False, debug=False,
                   num_devices=NCORES)
    xt_d = nc.dram_tensor("xT", [D, SQ], F16, kind="ExternalInput")
    ct_d = nc.dram_tensor("cT", [D, SK], F16, kind="ExternalInput")
    wq_d = nc.dram_tensor("wq", [D, DG], F16, kind="ExternalInput")
    wk_d = nc.dram_tensor("wk", [D, DG], F16, kind="ExternalInput")
    wv_d = nc.dram_tensor("wv", [D, DG], F16, kind="ExternalInput")
    wo_d = nc.dram_tensor("wo", [DG, D], F16, kind="ExternalInput")
    bq_d = nc.dram_tensor("bq", [DG], F32, kind="ExternalInput")
    bk_d = nc.dram_tensor("bk", [DG], F32, kind="ExternalInput")
    out_d = nc.dram_tensor("out", [SQ, D], F32, kind="ExternalOutput")

    with tile.TileContext(nc) as tc:
        with (
            tc.tile_pool(name="const", bufs=1) as cp,
            tc.tile_pool(name="persist", bufs=1) as psb,
            tc.tile_pool(name="expp", bufs=34) as ep,
            tc.tile_pool(name="rcpool", bufs=4) as rcp,
            tc.tile_pool(name="outp", bufs=2) as opool,
            tc.tile_pool(name="stp", bufs=2, space="PSUM") as stp,
            tc.tile_pool(name="atp", bufs=2, space="PSUM") as atp,
            tc.tile_pool(name="pp", bufs=2, space="PSUM") as pp,
        ):
            import contextlib
            loop_ctx = tc.For_i(0, loop_iters, 1) if loop_iters else contextlib.nullcontext()
            loop_ctx.__enter__()

            xt_sb = cp.tile([128, 8, SQ], F16, tag="xt")
            ct_sb = cp.tile([128, 8, SK], F16, tag="ct")
            wq_sb = cp.tile([128, 8, DG], F16, tag="wq")
            wk_sb = cp.tile([128, 8, DG], F16, tag="wk")
            wv_sb = cp.tile([128, 8, DG], F16, tag="wv")
            wo_sb = cp.tile([128, 2, D], F16, tag="wo")
            bq_sb = cp.tile([128, 2], F32, tag="bq")
            bk_sb = cp.tile([128, 2], F32, tag="bk")


            def load_xt(s4):
                nc.sync.dma_start(
                    out=xt_sb[:, :, s4 * 512:(s4 + 1) * 512],
                    in_=xt_d[:, s4 * 512:(s4 + 1) * 512]
                    .rearrange("(c p) s -> p c s", p=128))

            def load_ct(s4):
                nc.sync.dma_start(
                    out=ct_sb[:, :, s4 * 512:(s4 + 1) * 512],
                    in_=ct_d[:, s4 * 512:(s4 + 1) * 512]
                    .rearrange("(c p) s -> p c s", p=128))

            def load_all():
                # ordered so the first compute units unblock earliest
                nc.sync.dma_start(out=wk_sb, in_=wk_d[:].rearrange("(c p) n -> p c n", p=128))
                nc.sync.dma_start(out=bk_sb, in_=bk_d[:].rearrange("(c p) -> p c", p=128))
                load_ct(0)
                nc.sync.dma_start(out=wq_sb, in_=wq_d[:].rearrange("(c p) n -> p c n", p=128))
                nc.sync.dma_start(out=bq_sb, in_=bq_d[:].rearrange("(c p) -> p c", p=128))
                load_xt(0)
                load_xt(1)
                load_ct(1)
                nc.sync.dma_start(out=wv_sb, in_=wv_d[:].rearrange("(c p) n -> p c n", p=128))
                load_xt(2)
                load_xt(3)
                load_ct(2)
                load_ct(3)
                nc.sync.dma_start(out=wo_sb, in_=wo_d[:].rearrange("(c p) n -> p c n", p=128))

            # persistent activations
            qT = psb.tile([128, 2, SQ], F16, tag="qT", name="qT")
            kT = psb.tile([128, 2, SK], F16, tag="kT", name="kT")
            # v natural, 4 window tiles of 4 sk-chunks; col 64 = ones
            vAs = [psb.tile([128, 4, HG, 68], F16, tag=f"vA{w}", name=f"vA{w}")
                   for w in range(4)]
            aT = psb.tile([128, 2, SQ], F16, tag="aT", name="aT")

            for w in range(4):
                nc.vector.memset(vAs[w][:], 1.0)

            # ---- projection units ----
            def proj_qk(src_sb, w_sb, bias_sb, dst, c, s4):
                pq = atp.tile([128, 512], F32, tag="at")
                for dc in range(8):
                    nc.tensor.matmul(
                        pq,
                        (w_sb[:, dc, c * 128:(c + 1) * 128]),
                        (src_sb[:, dc, s4 * 512:(s4 + 1) * 512]),
                        start=(dc == 0), stop=(dc == 7),
                    )
                nc.vector.tensor_scalar_add(
                    dst[:, c, s4 * 512:(s4 + 1) * 512], pq, bias_sb[:, c:c + 1])

            def proj_q(c, s4):
                proj_qk(xt_sb, wq_sb, bq_sb, qT, c, s4)

            def proj_k(c, s4):
                proj_qk(ct_sb, wk_sb, bk_sb, kT, c, s4)

            def proj_v(skc):
                pv = pp.tile([128, 512], F32, tag="pp")
                for dc in range(8):
                    nc.tensor.matmul(
                        pv[:, 0:DG],
                        (ct_sb[:, dc, skc * 128:(skc + 1) * 128]),
                        (wv_sb[:, dc, :]),
                        start=(dc == 0), stop=(dc == 7),
                    )
                nc.vector.tensor_copy(
                    vAs[skc // 4][:, skc % 4, :, 0:64],
                    pv[:, 0:DG].rearrange("p (h e) -> p h e", e=64),
                )

            # ---- scores + exp ----
            exd = {}

            def sc(h, sqw, skc):
                p0 = 64 * (h % 2)
                t = h // 2
                st = stp.tile([128, 1024], F32, tag="st")
                for half in range(2):
                    nc.tensor.matmul(
                        st[:, half * 512:(half + 1) * 512],
                        (kT[p0:p0 + 64, t, skc * 128:(skc + 1) * 128]),
                        (qT[p0:p0 + 64, t,
                            sqw * 1024 + half * 512:sqw * 1024 + (half + 1) * 512]),
                        start=True, stop=True,
                    )
                ex = ep.tile([128, 1024], F16, tag="ex")
                nc.scalar.activation(ex, st, mybir.ActivationFunctionType.Exp,
                                     scale=SCALE)
                exd[(h, sqw, skc)] = ex

            # ---- attention half-window: psum attnT [65, 512] ----
            def ah(h, sqw, half):
                p0 = 64 * (h % 2)
                t = h // 2
                at = atp.tile([128, 512], F32, tag="at")
                for skc in range(16):
                    nc.tensor.matmul(
                        at[0:65, :],
                        (vAs[skc // 4][:, skc % 4, h, 0:65]),
                        (exd[(h, sqw, skc)][:, half * 512:(half + 1) * 512]),
                        start=(skc == 0), stop=(skc == 15),
                    )
                rc = rcp.tile([1, 512], F32, tag="rc")
                nc.vector.reciprocal(rc, at[64:65, :])
                rcb = rcp.tile([64, 512], F32, tag="rcb")
                nc.gpsimd.partition_broadcast(rcb[:], rc[:])
                nc.vector.tensor_tensor(
                    aT[p0:p0 + 64, t,
                       sqw * 1024 + half * 512:sqw * 1024 + (half + 1) * 512],
                    at[0:64, :],
                    rcb[:],
                    op=mybir.AluOpType.mult,
                )

            def attn(h, sqw):
                ah(h, sqw, 0)
                ah(h, sqw, 1)

            # ---- out projection: psum -> DRAM DMA directly ----
            def op(sqc):
                pos = [pp.tile([128, 512], F32, tag="pp", name=f"po{sqc}_{i}")
                       for i in range(2)]
                for kc in range(2):
                    for n2 in range(2):
                        nc.tensor.matmul(
                            pos[n2],
                            (aT[:, kc, sqc * 128:(sqc + 1) * 128]),
                            (wo_sb[:, kc, n2 * 512:(n2 + 1) * 512]),
                            start=(kc == 0), stop=(kc == 1),
                        )
                ot = opool.tile([128, D], F32, tag="ot")
                nc.vector.tensor_copy(ot[:, 0:512], pos[0])
                nc.vector.tensor_copy(ot[:, 512:1024], pos[1])
                nc.sync.dma_start(
                    out=out_d[sqc * 128:(sqc + 1) * 128, :], in_=ot)

            if pipeline:
                # startup: q c0 (sqw0 heads 0/1), k c0, scores unit (0,0)
                load_all()
                proj_k(0, 0)
                proj_q(0, 0)
                proj_q(0, 1)
                sc(0, 0, 0); sc(0, 0, 1)
                proj_k(0, 1)
                sc(0, 0, 2); sc(0, 0, 3)
                proj_k(0, 2)
                sc(0, 0, 4); sc(0, 0, 5)
                proj_k(0, 3)
                sc(0, 0, 6); sc(0, 0, 7)
                proj_q(1, 0)
                sc(0, 0, 8); sc(0, 0, 9)
                proj_q(1, 1)
                sc(0, 0, 10); sc(0, 0, 11)
                proj_k(1, 0)
                sc(0, 0, 12); sc(0, 0, 13)
                proj_k(1, 1)
                sc(0, 0, 14); sc(0, 0, 15)
                # v block, woven with unit (1,0) scores to keep ACT fed
                for i in range(16):
                    proj_v(i)
                    sc(1, 0, i)
                    if i == 7:
                        proj_k(1, 2)
                    if i == 11:
                        proj_k(1, 3)

                # steady state: scores(u) interleaved with attn(prev u)
                units = [(2, 0), (3, 0), (0, 1), (1, 1), (2, 1), (3, 1)]
                prevs = [(0, 0), (1, 0), (2, 0), (3, 0), (0, 1), (1, 1)]
                # weave: projections for sqw1 of q, then out_proj chunks 0-7
                pre_weave = {2: [lambda: proj_q(0, 2), lambda: proj_q(0, 3)],
                             3: [lambda: proj_q(1, 2), lambda: proj_q(1, 3)]}
                mid_weave = {3: [lambda: op(0)],
                             4: [lambda: op(1), lambda: op(2)],
                             5: [lambda: op(3), lambda: op(4), lambda: op(5),
                                 lambda: op(6), lambda: op(7)]}
                for i, (u, p) in enumerate(zip(units, prevs)):
                    h, sqw = u
                    for w in pre_weave.get(i, []):
                        w()
                    for skc in range(0, 4):
                        sc(h, sqw, skc)
                    ah(*p, 0)
                    for skc in range(4, 8):
                        sc(h, sqw, skc)
                    ws = mid_weave.get(i, [])
                    for w in ws[:2]:
                        w()
                    for skc in range(8, 12):
                        sc(h, sqw, skc)
                    ah(*p, 1)
                    for skc in range(12, 16):
                        sc(h, sqw, skc)
                    for w in ws[2:]:
                        w()
                # tail: ops 8-11 only need sq 1024-1535 (half 0) of all
                # sqw1 heads; 12-15 need half 1 -- interleave accordingly
                ah(2, 1, 0)
                ah(3, 1, 0)
                op(8); op(9)
                ah(2, 1, 1)
                op(10); op(11)
                ah(3, 1, 1)
                op(12); op(13); op(14); op(15)
            else:
                load_all()
                for c in range(2):
                    for s4 in range(4):
                        proj_q(c, s4)
                        proj_k(c, s4)
                for skc in range(16):
                    proj_v(skc)
                for sqw in range(2):
                    for h in range(HG):
                        for skc in range(16):
                            sc(h, sqw, skc)
                        attn(h, sqw)
                for sqc in range(16):
                    op(sqc)
            loop_ctx.__exit__(None, None, None)

    nc.compile()
    return nc


_NC = None


def _program():
    global _NC
    if _NC is None:
        _NC = build_program()
    return _NC


def _f32(a):
    return np.ascontiguousarray(np.asarray(a, dtype=np.float32))


def make_in_maps(inputs, context, Wq, bq, Wk, bk, Wv, bv, Wo, bo):
    in_maps = []
    for core in range(NCORES):
        b, g = core // HG, core % HG
        sl = slice(DG * g, DG * (g + 1))
        in_maps.append({
            "xT": np.ascontiguousarray(inputs[b].T.astype(np.float16)),
            "cT": np.ascontiguousarray(context[b].T.astype(np.float16)),
            "wq": np.ascontiguousarray(Wq[:, sl].astype(np.float16)),
            "wk": np.ascontiguousarray(Wk[:, sl].astype(np.float16)),
            "wv": np.ascontiguousarray(Wv[:, sl].astype(np.float16)),
            "wo": np.ascontiguousarray(Wo[sl, :].astype(np.float16)),
            "bq": _f32(bq[sl]),
            "bk": _f32(bk[sl]),
        })
    return in_maps


def kernel(inputs, context, Wq, bq, Wk, bk, Wv, bv, Wo, bo):
    from concourse.bass_utils import run_bass_kernel_spmd

    inputs = _f32(inputs)
    context = _f32(context)
    Wq, bq, Wk, bk = _f32(Wq), _f32(bq), _f32(Wk), _f32(bk)
    Wv, bv, Wo, bo = _f32(Wv), _f32(bv), _f32(Wo), _f32(bo)

    nc = _program()
    in_maps = make_in_maps(inputs, context, Wq, bq, Wk, bk, Wv, bv, Wo, bo)
    res = run_bass_kernel_spmd(nc, in_maps, list(range(NCORES)))
    outs = [res.results[i]["out"] for i in range(NCORES)]
    corr = (bv.astype(np.float64) @ Wo.astype(np.float64)
            + bo.astype(np.float64)).astype(np.float32)
    full = np.stack([
        outs[0] + outs[1] + outs[2] + outs[3],
        outs[4] + outs[5] + outs[6] + outs[7],
    ]) + corr
    return full.astype(np.float32)
